# revision 6
# baseline (speedup 1.0000x reference)
"""Fused multi-head self-attention (T=2048, B=2, E=1024, H=16) on 8 TRN2 cores.

Sharding: batch*heads across cores — core c handles b = c//4, heads
[(c%4)*4, (c%4)*4+4). Projections are column-split (Wq/Wk/Wv) per core's
heads; Wo is row-split with the cross-core reduction done on the host
during unshard (4 partial [T,E] sums per batch element).

Device kernel (per core, identical SPMD program):
  - qT/kT produced transposed [64*2-pair, T] so scores need no transposes
  - scores computed transposed sT[s,m] = kT.T @ qT, softmax along the
    PSUM free dim is avoided entirely: exp on ScalarE, denominators via a
    ones-column appended to v (row 64 of the AV accumulation), normalize
    by a K=1 broadcast matmul + DVE multiply
  - causal structure exploited by block classification (compile-time):
    fully-masked 128x128 blocks skipped, zero blocks exp'd directly,
    additive blocks get the real mask values added
  - all matmuls in float32r (same accuracy as fp32 matmul on TRN2 HW,
    4x the throughput)
"""
import os
import sys

import numpy as np

for _p in ("/opt/trn_rl_repo", "/root/.axon_site/_ro/trn_rl_repo"):
    if os.path.isdir(_p) and _p not in sys.path:
        sys.path.insert(0, _p)
        break

import concourse.bacc as bacc
import concourse.mybir as mybir
import concourse.tile as tile
from concourse.bass_utils import run_bass_kernel_spmd

f32 = mybir.dt.float32
f32r = mybir.dt.float32r
AF = mybir.ActivationFunctionType

T, B, E, H, HD = 2048, 2, 1024, 16, 64
NCORES = 8
HL = (B * H) // NCORES          # heads per core = 4
J = HL * HD                     # per-core projection width = 256
EC = E // 128                   # e-chunks = 8
SCALE = HD ** -0.5
MCH = 512                       # m-chunk width
NEG_THRESH = -1e8               # "fully masked" threshold

SKIP, ZERO, ADD = 0, 1, 2

_prog_cache = {}


def _classify_mask(mask):
    """Classify 128x128 blocks of mask[t_query, s_key]."""
    nb = mask.shape[0] // 128
    blocks = mask.reshape(nb, 128, nb, 128)
    all_skip = (blocks <= NEG_THRESH).all(axis=(1, 3))
    all_zero = (blocks == 0.0).all(axis=(1, 3))
    cls = np.where(all_skip, SKIP, np.where(all_zero, ZERO, ADD))
    return cls  # [m_block, s_block]


def _build(T_, cls_key):
    cls = np.array(cls_key, dtype=np.int64)
    NB = T_ // 128
    NMC = T_ // MCH
    add_blocks = [(mb, sb) for mb in range(NB) for sb in range(NB)
                  if cls[mb, sb] == ADD]
    add_pos = {blk: i for i, blk in enumerate(add_blocks)}
    n_add = len(add_blocks)

    nc = bacc.Bacc("TRN2", target_bir_lowering=False, debug=False)
    xT = nc.declare_dram_parameter("xT", [E, T_], f32, isOutput=False)
    wq = nc.declare_dram_parameter("wq", [E, J], f32, isOutput=False)
    wk = nc.declare_dram_parameter("wk", [E, J], f32, isOutput=False)
    wv = nc.declare_dram_parameter("wv", [E, J], f32, isOutput=False)
    wo = nc.declare_dram_parameter("wo", [J, E], f32, isOutput=False)
    bqp = nc.declare_dram_parameter("bqp", [128, 2], f32, isOutput=False)
    ones1 = nc.declare_dram_parameter("ones1", [1, 64], f32, isOutput=False)
    onescol = nc.declare_dram_parameter("onescol", [128, HL * NB], f32,
                                        isOutput=False)
    msk = nc.declare_dram_parameter("msk", [128, max(n_add, 1) * 128], f32,
                                    isOutput=False)
    out = nc.declare_dram_parameter("out", [T_, E], f32, isOutput=True)

    with tile.TileContext(nc) as tc:
        with nc.allow_low_precision(reason="float32r tiles are fp32-width"), \
             tc.tile_pool(name="sba", bufs=1) as sba, \
             tc.tile_pool(name="sbw", bufs=1) as sbw, \
             tc.tile_pool(name="ps", bufs=1, space="PSUM") as ps:
            xT_sb = sba.tile([128, EC * T_], f32r)
            wq_sb = sba.tile([128, EC * J], f32r)
            wk_sb = sba.tile([128, EC * J], f32r)
            wv_sb = sba.tile([128, EC * J], f32r)
            wo_sb = sba.tile([128, (J // 128) * E], f32r)
            qT_sb = sba.tile([128, 2 * T_], f32r)
            kT_sb = sba.tile([128, 2 * T_], f32r)
            v_sb = sba.tile([128, HL * NB * 65], f32r)
            oT_sb = sba.tile([128, 2 * T_], f32r)
            bq_sb = sba.tile([128, 2], f32)
            msk_sb = sba.tile([128, max(n_add, 1) * 128], f32)
            ones1_sb = sba.tile([1, 64], f32r)
            negc = sba.tile([128, 1], f32)
            nc.vector.memset(negc[:], -100.0)

            # ---- input DMAs ----
            for c in range(EC):
                nc.sync.dma_start(xT_sb[:, c * T_:(c + 1) * T_],
                                  xT[c * 128:(c + 1) * 128, :].bitcast(f32r))
            for wsb, wdr in ((wq_sb, wq), (wk_sb, wk), (wv_sb, wv)):
                nc.sync.dma_start(
                    wsb[:].rearrange("p (c j) -> p c j", j=J),
                    wdr[:, :].rearrange("(c p) j -> p c j", p=128).bitcast(f32r))
            nc.sync.dma_start(
                wo_sb[:].rearrange("p (c e) -> p c e", e=E),
                wo[:, :].rearrange("(c p) e -> p c e", p=128).bitcast(f32r))
            nc.sync.dma_start(bq_sb[:], bqp[:, :])
            nc.sync.dma_start(ones1_sb[:], ones1[:, :].bitcast(f32r))
            v_ones_view = v_sb[:].rearrange("p (x c) -> p x c", c=65)[:, :, 64:65]
            nc.sync.dma_start(v_ones_view, onescol[:, :].bitcast(f32r))
            if n_add:
                nc.sync.dma_start(msk_sb[:], msk[:, :])

            # ---- q/k projections (transposed layout [pair*128, T]) ----
            for n in range(T_ // 512):
                for u in range(2):
                    for wsb, dst, biased in ((wq_sb, qT_sb, True),
                                             (wk_sb, kT_sb, False)):
                        psq = ps.tile([128, 512], f32, tag="proj", bufs=2)
                        for c in range(EC):
                            nc.tensor.matmul(
                                psq[:],
                                wsb[:, c * J + u * 128: c * J + (u + 1) * 128],
                                xT_sb[:, c * T_ + n * 512: c * T_ + n * 512 + 512],
                                start=(c == 0), stop=(c == EC - 1))
                        dslc = dst[:, u * T_ + n * 512: u * T_ + n * 512 + 512]
                        if biased:
                            nc.vector.tensor_scalar_add(dslc, psq[:],
                                                        bq_sb[:, u:u + 1])
                        else:
                            nc.vector.tensor_copy(dslc, psq[:])

            # ---- v projection ([t, j] layout, 65-col strips with ones) ----
            for i in range(NB):
                psv = ps.tile([128, 512], f32, tag="proj", bufs=2)
                for c in range(EC):
                    nc.tensor.matmul(
                        psv[:, 0:J],
                        xT_sb[:, c * T_ + i * 128: c * T_ + i * 128 + 128],
                        wv_sb[:, c * J:(c + 1) * J],
                        start=(c == 0), stop=(c == EC - 1))
                for h in range(HL):
                    nc.vector.tensor_copy(
                        v_sb[:, (h * NB + i) * 65:(h * NB + i) * 65 + 64],
                        psv[:, h * 64:(h + 1) * 64])

            # ---- attention + output projection, per m-chunk ----
            for n in range(NMC):
                for h in range(HL):
                    u, poff = h >> 1, (h & 1) * 64
                    stiles = [i for i in range(NB)
                              if any(cls[n * 4 + k, i] != SKIP for k in range(4))]
                    pso_ = ps.tile([128, 512], f32, tag="attno", bufs=2)
                    for idx, i in enumerate(stiles):
                        pss = ps.tile([128, 512], f32, tag="attns", bufs=2)
                        nc.tensor.matmul(
                            pss[:],
                            kT_sb[poff:poff + 64, u * T_ + i * 128: u * T_ + i * 128 + 128],
                            qT_sb[poff:poff + 64, u * T_ + n * 512: u * T_ + n * 512 + 512],
                            start=True, stop=True, skip_group_check=True)
                        for k in range(4):
                            if cls[n * 4 + k, i] == ADD:
                                pos = add_pos[(n * 4 + k, i)]
                                nc.vector.tensor_add(
                                    pss[:, k * 128:(k + 1) * 128],
                                    pss[:, k * 128:(k + 1) * 128],
                                    msk_sb[:, pos * 128:(pos + 1) * 128])
                        pt = sbw.tile([128, 512], f32r, tag="pt", bufs=3)
                        # exp over runs of equal skip-ness
                        k = 0
                        while k < 4:
                            k1 = k
                            skipk = cls[n * 4 + k, i] == SKIP
                            while k1 < 4 and (cls[n * 4 + k1, i] == SKIP) == skipk:
                                k1 += 1
                            src = pss[:, k * 128:k1 * 128]
                            dst = pt[:, k * 128:k1 * 128]
                            if skipk:
                                nc.scalar.activation(dst, src, AF.Exp,
                                                     scale=0.0, bias=negc[:])
                            else:
                                nc.scalar.activation(dst, src, AF.Exp)
                            k = k1
                        nc.tensor.matmul(
                            pso_[0:65, :],
                            v_sb[:, (h * NB + i) * 65:(h * NB + i) * 65 + 65],
                            pt[:],
                            start=(idx == 0), stop=(idx == len(stiles) - 1),
                            skip_group_check=True)
                    recip = sbw.tile([1, 512], f32r, tag="recip", bufs=2)
                    nc.vector.reciprocal(recip[:], pso_[64:65, :])
                    psb = ps.tile([64, 512], f32, tag="attnb", bufs=1)
                    nc.tensor.matmul(psb[:], ones1_sb[:], recip[:],
                                     start=True, stop=True, skip_group_check=True)
                    rb = sbw.tile([64, 512], f32, tag="rb", bufs=2)
                    nc.vector.tensor_copy(rb[:], psb[:])
                    nc.vector.tensor_mul(
                        oT_sb[poff:poff + 64, u * T_ + n * 512: u * T_ + n * 512 + 512],
                        pso_[0:64, :], rb[:])
                # output projection for this m-chunk's four t-tiles
                for m16 in range(n * 4, n * 4 + 4):
                    for eh in range(E // 512):
                        pso = ps.tile([128, 512], f32, tag="attnout", bufs=1)
                        for jc in range(J // 128):
                            nc.tensor.matmul(
                                pso[:],
                                oT_sb[:, jc * T_ + m16 * 128: jc * T_ + m16 * 128 + 128],
                                wo_sb[:, jc * E + eh * 512: jc * E + eh * 512 + 512],
                                start=(jc == 0), stop=(jc == J // 128 - 1),
                                skip_group_check=True)
                        ob = sbw.tile([128, 512], f32, tag="ob", bufs=3)
                        nc.vector.tensor_copy(ob[:], pso[:])
                        nc.sync.dma_start(
                            out[m16 * 128:(m16 + 1) * 128,
                                eh * 512:(eh + 1) * 512], ob[:])

    nc.compile()
    return nc


def _get_program(T_, cls):
    key = (T_, tuple(map(tuple, cls.tolist())))
    if key not in _prog_cache:
        _prog_cache[key] = _build(T_, key[1])
    return _prog_cache[key]


def _numpy_ref(query, attn_mask, key_padding_mask, Wq, bq, Wk, bk, Wv, bv,
               Wo, bo):
    """Exact-semantics fallback (mirrors reference.py in numpy)."""
    q = (query @ Wq.T + bq) * SCALE
    k = query @ Wk.T + bk
    v = query @ Wv.T + bv

    def shp(x):
        return x.reshape(T, B * H, HD).transpose(1, 0, 2)

    q, k, v = shp(q), shp(k), shp(v)
    w = np.einsum('bth,bsh->bts', q, k).reshape(B, H, T, T) + attn_mask
    w = np.where(key_padding_mask[:, None, None, :], -np.inf, w)
    w = w - w.max(axis=-1, keepdims=True)
    ew = np.exp(w)
    p = (ew / ew.sum(axis=-1, keepdims=True)).reshape(B * H, T, T)
    o = np.einsum('bts,bsh->bth', p, v.reshape(B * H, T, HD))
    o = o.transpose(1, 0, 2).reshape(T, B, E)
    return (o @ Wo.T + bo).astype(np.float32)


def _prep_inputs(query, attn_mask, Wq, bq, Wk, Wv, Wo, cls):
    """Build the 8 per-core input maps."""
    add_blocks = [(mb, sb) for mb in range(T // 128) for sb in range(T // 128)
                  if cls[mb, sb] == ADD]
    n_add = len(add_blocks)
    if n_add:
        mskp = np.empty((128, n_add * 128), np.float32)
        for i, (mb, sb) in enumerate(add_blocks):
            blk = attn_mask[mb * 128:(mb + 1) * 128, sb * 128:(sb + 1) * 128]
            mskp[:, i * 128:(i + 1) * 128] = np.ascontiguousarray(blk.T)
    else:
        mskp = np.zeros((128, 128), np.float32)
    ones1 = np.ones((1, 64), np.float32)
    onescol = np.ones((128, HL * (T // 128)), np.float32)

    in_maps = []
    for core in range(NCORES):
        b = core // (NCORES // B)
        jsl = slice((core % (NCORES // B)) * J, (core % (NCORES // B)) * J + J)
        xT_c = np.ascontiguousarray(query[:, b, :].T)
        wq_c = np.ascontiguousarray((Wq[jsl, :] * np.float32(SCALE)).T)
        wk_c = np.ascontiguousarray(Wk[jsl, :].T)
        wv_c = np.ascontiguousarray(Wv[jsl, :].T)
        wo_c = np.ascontiguousarray(Wo[:, jsl].T)
        bq_c = np.ascontiguousarray(
            (bq[jsl] * np.float32(SCALE)).reshape(2, 128).T)
        in_maps.append({
            "xT": xT_c, "wq": wq_c, "wk": wk_c, "wv": wv_c, "wo": wo_c,
            "bqp": bq_c, "ones1": ones1, "onescol": onescol, "msk": mskp,
        })
    return in_maps


def _kernel_impl(inputs, trace=False, **run_kwargs):
    query = np.asarray(inputs["query"], np.float32)
    attn_mask = np.asarray(inputs["attn_mask"], np.float32)
    kpm = np.asarray(inputs["key_padding_mask"])
    Wq = np.asarray(inputs["Wq"], np.float32)
    bq = np.asarray(inputs["bq"], np.float32)
    Wk = np.asarray(inputs["Wk"], np.float32)
    bk = np.asarray(inputs["bk"], np.float32)
    Wv = np.asarray(inputs["Wv"], np.float32)
    bv = np.asarray(inputs["bv"], np.float32)
    Wo = np.asarray(inputs["Wo"], np.float32)
    bo = np.asarray(inputs["bo"], np.float32)

    # Fast path requires: no key padding, no fully-masked rows, block-
    # classifiable mask with a modest number of additive blocks, and no
    # bk dependence issue (bk shifts are softmax-invariant, always ok).
    cls = _classify_mask(attn_mask)
    fallback = (
        kpm.any()
        or (attn_mask.max(axis=1) <= NEG_THRESH).any()
        or (cls == ADD).sum() > 24
        or np.isnan(attn_mask).any()
    )
    if fallback:
        return _numpy_ref(query, attn_mask, kpm, Wq, bq, Wk, bk, Wv, bv,
                          Wo, bo), None

    nc = _get_program(T, cls)
    in_maps = _prep_inputs(query, attn_mask, Wq, bq, Wk, Wv, Wo, cls)
    res = run_bass_kernel_spmd(nc, in_maps, core_ids=list(range(NCORES)),
                               trace=trace, **run_kwargs)

    # unshard: sum the 4 row-split partials per batch element (the Wo
    # all-reduce), then add bo and the bv contribution (sum_s p = 1).
    bo_total = bo + Wo @ bv
    out = np.empty((T, B, E), np.float32)
    gsz = NCORES // B
    for b in range(B):
        acc = res.results[b * gsz]["out"].astype(np.float32)
        for c in range(b * gsz + 1, (b + 1) * gsz):
            acc = acc + res.results[c]["out"]
        out[:, b, :] = acc + bo_total[None, :]
    return out, res


def kernel(**inputs):
    out, _ = _kernel_impl(inputs, trace=False)
    return out
